# revision 37
# baseline (speedup 1.0000x reference)
"""Multi-head attention (B=8, S=2048, D=1024, H=16, DK=64) on 8 TRN2 NeuronCores.

Sharding: pure batch data-parallel — core i computes batch i's full attention.
No collectives needed; per-core output is the final [S, D] slice.

Per-core pipeline (all matmuls bf16, fp32 PSUM accumulation):
  1. gpsimd cast-DMA inputs f32->bf16 into DRAM staging, then HW DMA-transpose
     loads to get qT/kT/vT in [D, S] SBUF layout (contraction dim on partitions).
  2. Projections with head-PAIR packed weights: lhsT = [d, 2*64] so one matmul
     yields two heads' projected rows. q/k projected transposed [dk, s]; v
     projected natural [t, dk] with a ones column appended (softmax denominator
     comes out of the attention*V matmul for free).
  3. Scores computed transposed: scoresT[t, s] = kT_h.T @ qT_h, two heads
     row-packed into array rows 0-63 / 64-127 (K=64 each, concurrent).
  4. exp((1/32)*x) fused on ScalarE reading PSUM [128, 1024], writing bf16.
  5. AV: lhsT = [v_h | ones] [t, 65] -> out rows 0-63 = out_hT, row 64 = denom.
  6. normalize: reciprocal(denom) -> gpsimd partition_broadcast -> multiply;
     final Wo projection from transposed out tiles, interleaved with the next
     s-chunk's attention to keep ScalarE fed.
"""

import sys

if "/opt/trn_rl_repo" not in sys.path:
    sys.path.insert(0, "/opt/trn_rl_repo")

import functools
from contextlib import ExitStack

import numpy as np

import concourse.bass as bass
import concourse.mybir as mybir
import concourse.tile as tile
from concourse import bacc
from concourse.bass_utils import run_bass_kernel_spmd

F32 = mybir.dt.float32
BF16 = mybir.dt.bfloat16
P = 128

B, D, H, DK = 8, 1024, 16, 64
S_FULL = 2048
NPAIR = H // 2  # 8 head pairs
DT = D // P  # 8 d-tiles (contraction tiles for projections)
N_CORES = 8


def _body(ctx: ExitStack, tc: tile.TileContext, S: int):
    nc = tc.nc
    TT = S // P  # t-tiles
    SCW = min(1024, S)  # attention s-chunk width
    SC = S // SCW  # number of s chunks
    W5 = min(512, S)  # matmul free-dim width (one PSUM bank)
    NH = SCW // W5  # W5-wide halves per chunk

    q_ap = nc.dram_tensor("q", [S, D], F32, kind="ExternalInput").ap()
    k_ap = nc.dram_tensor("k", [S, D], F32, kind="ExternalInput").ap()
    v_ap = nc.dram_tensor("v", [S, D], F32, kind="ExternalInput").ap()
    wq_ap = nc.dram_tensor("Wq", [H, D, DK], F32, kind="ExternalInput").ap()
    wk_ap = nc.dram_tensor("Wk", [H, D, DK], F32, kind="ExternalInput").ap()
    wv_ap = nc.dram_tensor("Wv", [H, D, DK], F32, kind="ExternalInput").ap()
    wo_ap = nc.dram_tensor("Wo", [D, D], F32, kind="ExternalInput").ap()
    out_ap = nc.dram_tensor("out", [S, D], F32, kind="ExternalOutput").ap()

    scale = float(D) ** -0.5

    dram = ctx.enter_context(tc.tile_pool(name="dram", bufs=1, space="DRAM"))
    consts = ctx.enter_context(tc.tile_pool(name="consts", bufs=1))
    wpool = ctx.enter_context(tc.tile_pool(name="wpool", bufs=2))
    res = ctx.enter_context(tc.tile_pool(name="res", bufs=1))
    # PSUM: "sc" = attention scores (2 x 2 banks); "av" = AV accumulators,
    # projections and the final Wo projection share it (2 x 2 banks).
    ps_sc = ctx.enter_context(tc.tile_pool(name="ps_sc", bufs=2, space="PSUM"))
    ps_av = ctx.enter_context(tc.tile_pool(name="ps_av", bufs=2, space="PSUM"))

    # ---- weights: pair-packed w[p, dt, pair, h2, dk] (bf16, cast in DMA);
    # loads are emitted just before their consumer so the gpsimd DMA queue
    # never delays the k staging chain ----
    def load_w(name, wap):
        wt = wpool.tile([P, DT, NPAIR, 2, DK], BF16, tag="w", name=name)
        srcw = wap.rearrange("h (dt p) k -> p dt h k", p=P)
        for dt_ in range(DT):
            nc.gpsimd.dma_start(wt[:, dt_], srcw[:, dt_])
        return wt

    # ---- inputs: stage bf16, transpose-load to [d, s] layout ----
    # chunked so each transpose only waits for its own staging rows
    def load_xT(xpool, x_ap, label, first_chunks=None):
        stage = dram.tile([S, D], BF16, tag=f"stage_{label}")
        n_chunks = max(1, S // 512)
        rows = S // n_chunks
        xT = xpool.tile([P, DT, S], BF16, tag="xT", name=f"{label}T")

        def emit(c):
            sl = slice(c * rows, (c + 1) * rows)
            nc.gpsimd.dma_start(stage[sl, :], x_ap[sl, :])
            # one 3D xbar transpose per chunk: xT[p, dt, s] = stage[s, dt*128+p]
            nc.sync.dma_start_transpose(xT[:, :, sl], stage[sl, :])

        if first_chunks is None:
            for c in range(n_chunks):
                emit(c)
            return xT
        first_chunks = min(first_chunks, n_chunks)
        for c in range(first_chunks):
            emit(c)

        def finish():
            for c in range(first_chunks, n_chunks):
                emit(c)

        return xT, finish

    # ---- projections (PSUM tiles on the "av" tag so the attention-score
    # pipeline's "sc" slots are free from the start) ----
    kproj = res.tile([P, NPAIR, S], BF16, tag="kproj")
    qproj = res.tile([P, NPAIR, S], BF16, tag="qproj")
    vaug = res.tile([P, H, TT, DK + 1], BF16, tag="vaug")
    nc.vector.memset(vaug[:, :, :, DK : DK + 1], 1.0)

    ones_sb = consts.tile([1, DK], BF16, tag="ones")
    nc.vector.memset(ones_sb[:], 1.0)
    # dummy exp: pulls the ~1.3us activation-table load into startup idle
    # instead of paying it before the first real exp on the critical path
    warm_sb = consts.tile([1, 32], F32, tag="warm")
    nc.vector.memset(warm_sb[:], 0.0)
    nc.scalar.activation(
        warm_sb[:], warm_sb[:], mybir.ActivationFunctionType.Exp, scale=1.0
    )

    def project_T(xT, w, dst, pools=None):
        # dst[h2*64+dk, pair, s] = sum_d w[d, pair, h2, dk] * xT[d, s]
        # scq outer: group (scq, pr) only needs transpose chunk scq, so the
        # PE consumption rate matches the staging+transpose feed rate.
        # k/v projections may also borrow the idle "sc" slots (pools list) —
        # they retire before q-proj ends, so attention scores never wait.
        if pools is None:
            pools = [(ps_av, "av")]
        gi = 0
        for scq in range(S // W5):
            for pr in range(NPAIR):
                pool_, tag_ = pools[gi % len(pools)]
                gi += 1
                ps = pool_.tile([P, W5], F32, tag=tag_, name="proj_ps")
                for dt_ in range(DT):
                    nc.tensor.matmul(
                        ps,
                        w[:, dt_, pr],
                        xT[:, dt_, scq * W5 : (scq + 1) * W5],
                        start=dt_ == 0,
                        stop=dt_ == DT - 1,
                    )
                nc.vector.tensor_copy(
                    out=dst[:, pr, scq * W5 : (scq + 1) * W5], in_=ps
                )

    with tc.tile_pool(name="xpool", bufs=2) as xpool:
        kT, finish_k = load_xT(xpool, k_ap, "k", first_chunks=1)
        wk = load_w("wk", wk_ap)
        finish_k()
        project_T(kT, wk, kproj, pools=[(ps_av, "av"), (ps_sc, "sc")])
        # v projected natural [t, h*dk]; half-major so heads 0-7 finish first
        vT, finish_v = load_xT(xpool, v_ap, "v", first_chunks=1)
        wv = load_w("wv", wv_ap)
        finish_v()
        vgi = 0
        for half in range(2):
            for tt in range(TT):
                vpool_, vtag_ = [(ps_av, "av"), (ps_sc, "sc")][vgi % 2]
                vgi += 1
                ps = vpool_.tile([P, 512], F32, tag=vtag_, name="vproj_ps")
                for dt_ in range(DT):
                    nc.tensor.matmul(
                        ps,
                        vT[:, dt_, tt * P : (tt + 1) * P],
                        wv[:, dt_, half * 4 : (half + 1) * 4],
                        start=dt_ == 0,
                        stop=dt_ == DT - 1,
                    )
                nc.vector.tensor_copy(
                    out=vaug[:, half * 8 : (half + 1) * 8, tt, 0:DK],
                    in_=ps.rearrange("p (h k) -> p h k", k=DK),
                )

        # q last: attention for pair 0 unblocks as soon as its first q
        # slices are projected, overlapping the rest of q-proj with attention
        qT, finish_q = load_xT(xpool, q_ap, "q", first_chunks=1)
        wq = load_w("wq", wq_ap)
        finish_q()
        project_T(qT, wq, qproj)

    # xpool released; woT lives in the reclaimed space (needed only once the
    # first s-chunk finishes)
    res2 = ctx.enter_context(tc.tile_pool(name="res2", bufs=1))
    woT = res2.tile([P, DT, D], BF16, tag="woT")
    wo_stage = dram.tile([D, D], BF16, tag="wo_stage")
    nc.gpsimd.dma_start(wo_stage[:], wo_ap)
    nc.sync.dma_start_transpose(woT[:], wo_stage[:])

    apool = ctx.enter_context(tc.tile_pool(name="apool", bufs=14))
    spool = ctx.enter_context(tc.tile_pool(name="spool", bufs=1))
    fpool = ctx.enter_context(tc.tile_pool(name="fpool", bufs=3))

    # ---- attention; the previous chunk's Wo projection is interleaved into
    # the pair loop so its PSUM/PE use rides along without starving ScalarE ----
    exp_f = mybir.ActivationFunctionType.Exp

    def final_proj_step(outT_prev, sc_prev, st, dcs=None):
        s0p = sc_prev * SCW
        for dc in range(D // W5) if dcs is None else dcs:
            f_ps = ps_av.tile([P, W5], F32, tag="av", name="f_ps")
            for kt in range(DT):
                nc.tensor.matmul(
                    f_ps,
                    outT_prev[:, kt, st * P : (st + 1) * P],
                    woT[:, kt, dc * W5 : (dc + 1) * W5],
                    start=kt == 0,
                    stop=kt == DT - 1,
                )
            fo = fpool.tile([P, W5], F32, tag="fo")
            nc.vector.tensor_copy(out=fo[:], in_=f_ps[:])
            nc.sync.dma_start(
                out_ap[s0p + st * P : s0p + (st + 1) * P, dc * W5 : (dc + 1) * W5],
                fo[:],
            )

    def emit_scores(sc_, pr, tt, boost=False):
        s0 = sc_ * SCW
        sc_ps = [
            ps_sc.tile([P, SCW], F32, tag="sc", name=f"sc{h2}") for h2 in range(2)
        ]
        for h2 in range(2):
            rows = slice(h2 * DK, (h2 + 1) * DK)
            lhsT = kproj[rows, pr, tt * P : (tt + 1) * P]
            for sh in range(NH):
                mm = nc.tensor.matmul(
                    sc_ps[h2][:, sh * W5 : (sh + 1) * W5],
                    lhsT,
                    qproj[rows, pr, s0 + sh * W5 : s0 + (sh + 1) * W5],
                )
                if boost:
                    # let the scheduler run the first attention unit's scores
                    # inside the tail of the q-projection instead of after it
                    mm.ins.bass_priority = -5
        return sc_ps

    outT_prev = None
    outT = None
    st_per_pair = max(1, (SCW // P) // NPAIR)  # final-proj subtiles per pair
    units = [(sc_, pr) for sc_ in range(SC) for pr in range(NPAIR)]
    for ui, (sc_, pr) in enumerate(units):
        if pr == 0:
            outT_prev = outT
            outT = wpool.tile([P, NPAIR, SCW], BF16, tag="w", name="outT")
        boost = ui == 0
        sc_ps = emit_scores(sc_, pr, 0, boost=boost)
        # weave the previous chunk's output projection: PSUM tiles allocated
        # at PAIR START (slots freed by the PREVIOUS pair's normalize), with
        # the matmuls emitted 2-at-a-time inside the tt loop so they ride the
        # PE's slack instead of ever blocking the next scores
        fps_steps = []
        if outT_prev is not None and st_per_pair == 1:
            st = pr
            s0p = (sc_ - 1) * SCW
            for dc in range(D // W5):
                f_ps = ps_av.tile([P, W5], F32, tag="av", name="f_ps")

                def mk(f_ps=f_ps, dc=dc, st=st, s0p=s0p):
                    def step(g):
                        for kt in (g, g + 1):
                            nc.tensor.matmul(
                                f_ps,
                                outT_prev[:, kt, st * P : (st + 1) * P],
                                woT[:, kt, dc * W5 : (dc + 1) * W5],
                                start=kt == 0,
                                stop=kt == DT - 1,
                            )
                        if g + 2 >= DT:
                            fo = fpool.tile([P, W5], F32, tag="fo")
                            nc.vector.tensor_copy(out=fo[:], in_=f_ps[:])
                            nc.sync.dma_start(
                                out_ap[
                                    s0p + st * P : s0p + (st + 1) * P,
                                    dc * W5 : (dc + 1) * W5,
                                ],
                                fo[:],
                            )
                    return step

                step = mk()
                for g in range(0, DT, 2):
                    fps_steps.append((step, g))
        av_ps = [
            ps_av.tile([DK + 1, SCW], F32, tag="av", name=f"av{h2}")
            for h2 in range(2)
        ]
        # software-pipelined: the next exp's scores (including the next
        # pair's first t-tile) are always emitted before AV / normalize /
        # final-proj matmuls, so ScalarE's next input is never queued
        # behind them on the PE
        for tt in range(TT):
            ats = []
            for h2 in range(2):
                at = apool.tile([P, SCW], BF16, tag="attn", name="at")
                ei = nc.scalar.activation(at[:], sc_ps[h2][:], exp_f, scale=scale)
                if boost:
                    ei.ins.bass_priority = -5
                ats.append(at)
            if tt + 1 < TT:
                sc_ps = emit_scores(sc_, pr, tt + 1, boost=boost)
            if tt < len(fps_steps):
                fps_steps[tt][0](fps_steps[tt][1])
            for h2 in range(2):
                va = vaug[:, 2 * pr + h2, tt, :]
                for sh in range(NH):
                    nc.tensor.matmul(
                        av_ps[h2][:, sh * W5 : (sh + 1) * W5],
                        va,
                        ats[h2][:, sh * W5 : (sh + 1) * W5],
                        start=tt == 0,
                        stop=tt == TT - 1,
                    )
        # normalize: out_hT = av[0:64] * (1 / av[64]) broadcast over rows
        for h2 in range(2):
            rec = spool.tile([1, SCW], F32, tag="rec")
            nc.vector.reciprocal(rec[:], av_ps[h2][DK : DK + 1, :])
            recb = spool.tile([1, SCW], BF16, tag="recb")
            nc.vector.tensor_copy(out=recb[:], in_=rec[:])
            bc_sb = spool.tile([DK, SCW], BF16, tag="bc_sb")
            nc.gpsimd.partition_broadcast(bc_sb[:], recb[:])
            nc.vector.tensor_tensor(
                outT[h2 * DK : (h2 + 1) * DK, pr, :],
                av_ps[h2][0:DK, :],
                bc_sb[:],
                mybir.AluOpType.mult,
            )
        if outT_prev is not None and st_per_pair != 1:
            for i in range(st_per_pair):
                st = pr * st_per_pair + i
                if st < SCW // P:
                    final_proj_step(outT_prev, sc_ - 1, st)

    for st in range(SCW // P):
        final_proj_step(outT, SC - 1, st)


@functools.lru_cache(maxsize=2)
def build(S: int = S_FULL):
    nc = bacc.Bacc("TRN2", target_bir_lowering=False, debug=False)
    with tile.TileContext(nc) as tc:
        with ExitStack() as ctx:
            _body(ctx, tc, S)
    nc.compile()
    return nc


def kernel(**inputs: np.ndarray) -> np.ndarray:
    query = np.ascontiguousarray(inputs["query"], dtype=np.float32)
    key = np.ascontiguousarray(inputs["key"], dtype=np.float32)
    value = np.ascontiguousarray(inputs["value"], dtype=np.float32)
    Wq = np.ascontiguousarray(inputs["Wq"], dtype=np.float32)
    Wk = np.ascontiguousarray(inputs["Wk"], dtype=np.float32)
    Wv = np.ascontiguousarray(inputs["Wv"], dtype=np.float32)
    Wo = np.ascontiguousarray(inputs["Wo"], dtype=np.float32)

    nc = build(S_FULL)
    in_maps = [
        {
            "q": query[i],
            "k": key[i],
            "v": value[i],
            "Wq": Wq,
            "Wk": Wk,
            "Wv": Wv,
            "Wo": Wo,
        }
        for i in range(N_CORES)
    ]
    res = run_bass_kernel_spmd(nc, in_maps, core_ids=list(range(N_CORES)))
    return np.stack([res.results[i]["out"] for i in range(N_CORES)], axis=0)


if __name__ == "__main__":
    rng = np.random.default_rng(0)
    ins = {
        "query": rng.standard_normal((B, S_FULL, D), dtype=np.float32),
        "key": rng.standard_normal((B, S_FULL, D), dtype=np.float32),
        "value": rng.standard_normal((B, S_FULL, D), dtype=np.float32),
        "Wq": rng.standard_normal((H, D, DK), dtype=np.float32) * 0.02,
        "Wk": rng.standard_normal((H, D, DK), dtype=np.float32) * 0.02,
        "Wv": rng.standard_normal((H, D, DK), dtype=np.float32) * 0.02,
        "Wo": rng.standard_normal((D, D), dtype=np.float32) * 0.02,
    }
    out = kernel(**ins)
    print(out.shape, out.dtype)
